# revision 16
# baseline (speedup 1.0000x reference)
"""Bass/Trainium2 kernel for nn_AvgPoolBackbone (segment_reduce).

Computes, for each batch row b of x [B, S, D]:
    eff = S if idx[b] == -1 else idx[b]
    out[b] = mean(x[b, :eff], axis=0)   (zeros when eff <= 0)

Strategy
--------
Pure data parallel over 8 NeuronCores (16 batches each).  On the host we
fold the prefix mask AND the 1/eff_len scaling into a single f32 matrix
`maskt` so the device does no division and no control flow.

Per batch, x[b] ([2048, 256] f32, 2 MiB) is viewed as [128, 16*256]:
partition p holds the 16 consecutive sequence rows p*16..p*16+15 — one
contiguous 16 KiB DRAM run per partition, which keeps the DMA descriptors
large.  The masked mean is then 16 PSUM-accumulated TensorE matmuls

    psum[1, D] += maskt[:, col].T @ x_view[:, j*D:(j+1)*D]

where maskt[p, col] = mask[b, p*16 + j] / eff_len[b].  Operands are
bitcast to float32r, which streams one PSUM row per cycle (4x faster
than the two-pass fp32 path) at N=256.  TensorE does the masking and the
cross-partition reduction in one instruction; the kernel is
HBM-bandwidth bound.
"""

import numpy as np

import concourse.bass as bass
import concourse.tile as tile
from concourse import bacc, mybir
from concourse import bass_utils

F32 = mybir.dt.float32
F32R = mybir.dt.float32r

# Problem config (hardcoded per the harness contract).
B, S, D = 128, 2048, 256
N_CORES = 8
BL = B // N_CORES  # batches per core
P = 128            # SBUF partitions
CHUNK_B = 2        # batches loaded per DMA


def build_kernel(bl=BL, s=S, d=D, chunk_b=CHUNK_B, f32r=False, dve_mod=2, bufs=6):
    """Build + compile the single-core Bass module (same NEFF on all cores).

    Batches alternate between two engines to halve the per-engine load
    while keeping exact fp32: batches with b % dve_mod == 0 run a DVE
    fused multiply-accumulate chain (then one PE ones-matmul folds the
    [128, d] partials across partitions); the other batches run 16
    PSUM-accumulated PE matmuls.  PSUM->SBUF result copies go to the
    otherwise idle ScalarE.  With f32r=True everything instead runs on
    PE in reduced-precision float32r (dve_mod ignored).
    """
    j = s // P  # seq rows per partition (16 at full size)
    mmdt = F32R if f32r else F32
    if f32r:
        dve_mod = 0
    nc = bacc.Bacc("TRN2", target_bir_lowering=False, debug=False)
    x = nc.dram_tensor("x", (bl, s, d), mmdt, kind="ExternalInput")
    maskt = nc.dram_tensor("maskt", (P, bl * j), mmdt, kind="ExternalInput")
    out = nc.dram_tensor("out", (1, bl * d), F32, kind="ExternalOutput")

    def is_dve(b):
        # odd batches on DVE so a PE batch leads the instruction stream
        return dve_mod > 0 and b % dve_mod == 1

    with tile.TileContext(nc) as tc:
        with (
            tc.tile_pool(name="xp", bufs=bufs) as xp,
            tc.tile_pool(name="mp", bufs=1) as mp,
            tc.tile_pool(name="op", bufs=1) as op,
            tc.tile_pool(name="ap", bufs=6) as apool,
            tc.tile_pool(name="ps", bufs=8, space=bass.MemorySpace.PSUM) as ps,
        ):
            m_t = mp.tile([P, bl * j], mmdt)
            nc.sync.dma_start(m_t[:], maskt.ap())
            ones_t = None
            if dve_mod > 0:
                ones_t = mp.tile([P, 1], F32)
                nc.vector.memset(ones_t[:], 1.0)
            o_t = op.tile([1, bl * d], F32)
            xv = x.ap().rearrange("b (p k) d -> p b (k d)", p=P)

            def emit_fold(b, acc_sb):
                # fold the DVE partials of batch b across partitions; the
                # call site defers this until PE has other work queued
                acc = ps.tile([1, d], F32)
                nc.tensor.matmul(
                    acc[:], ones_t[:], acc_sb[:], start=True, stop=True
                )
                nc.scalar.copy(o_t[:, b * d : (b + 1) * d], acc[:])

            def dve_chain(b, acc_sb, jis):
                for n, ji in enumerate(jis):
                    xs = x_tiles[b][:, ji * d : (ji + 1) * d]
                    mcol = m_t[:, b * j + ji : b * j + ji + 1]
                    if n == 0:
                        nc.vector.tensor_scalar_mul(acc_sb[:], xs, mcol)
                    else:
                        nc.vector.scalar_tensor_tensor(
                            acc_sb[:],
                            xs,
                            mcol,
                            acc_sb[:],
                            mybir.AluOpType.mult,
                            mybir.AluOpType.add,
                        )

            def pe_mms(b, acc, jis, start, stop):
                for n, ji in enumerate(jis):
                    nc.tensor.matmul(
                        acc[:],
                        m_t[:, b * j + ji : b * j + ji + 1],
                        x_tiles[b][:, ji * d : (ji + 1) * d],
                        start=(start and n == 0),
                        stop=(stop and n == len(jis) - 1),
                    )

            x_tiles = {}
            pending = None  # (batch, acc_sb) awaiting its fold matmul
            n_split = 2 if dve_mod > 0 else 0  # tail batches split PE/DVE
            for b in range(bl):
                # one 2 MiB DMA per batch on the sync HWDGE ring, in
                # consumption order; lands as [P, j*d] with one contiguous
                # 16 KiB DRAM run per partition
                x_t = xp.tile([P, j * d], mmdt)
                nc.sync.dma_start(x_t[:], xv[:, b])
                x_tiles[b] = x_t
                if b >= bl - n_split:
                    # tail: halve the post-last-DMA latency by giving each
                    # engine half the batch
                    if pending is not None:
                        emit_fold(*pending)
                        pending = None
                    acc_sb = apool.tile([P, d], F32)
                    dve_chain(b, acc_sb, range(j // 2))
                    acc = ps.tile([1, d], F32)
                    pe_mms(b, acc, range(j // 2, j), start=True, stop=False)
                    nc.tensor.matmul(
                        acc[:], ones_t[:], acc_sb[:], start=False, stop=True
                    )
                    nc.scalar.copy(o_t[:, b * d : (b + 1) * d], acc[:])
                elif is_dve(b):
                    acc_sb = apool.tile([P, d], F32)
                    dve_chain(b, acc_sb, range(j))
                    pending = (b, acc_sb)
                else:
                    acc = ps.tile([1, d], F32)
                    pe_mms(b, acc, range(j), start=True, stop=True)
                    nc.scalar.copy(o_t[:, b * d : (b + 1) * d], acc[:])
                    if pending is not None:
                        emit_fold(*pending)
                        pending = None
            if pending is not None:
                emit_fold(*pending)
            nc.sync.dma_start(out.ap(), o_t[:])

    nc.compile()
    return nc


def make_host_inputs(x, start_padding_indices, n_cores=N_CORES, bl=BL, s=S, d=D):
    """Shard x and build the per-core scaled mask matrices.

    maskt[p, b*j + ji] = (p*j + ji < eff[b]) / max(eff[b], 1)
    """
    x = np.ascontiguousarray(np.asarray(x, dtype=np.float32))
    idx = np.asarray(start_padding_indices).astype(np.int64)
    j = s // P
    eff = np.where(idx == -1, s, idx).astype(np.int64)  # [B]
    scale = 1.0 / np.maximum(eff, 1).astype(np.float64)
    mask = (np.arange(s)[None, :] < eff[:, None]) * scale[:, None]  # [B, S] f64
    mask = mask.astype(np.float32)
    # [B, S] -> [B, P, j] (s-major within partition) -> cores pack [P, bl*j]
    mask_pj = mask.reshape(-1, P, j)  # [B, P, j]
    in_maps = []
    for c in range(n_cores):
        mb = mask_pj[c * bl : (c + 1) * bl]  # [bl, P, j]
        maskt = np.ascontiguousarray(mb.transpose(1, 0, 2).reshape(P, bl * j))
        in_maps.append(
            {
                "x": np.ascontiguousarray(x[c * bl : (c + 1) * bl]),
                "maskt": maskt,
            }
        )
    return in_maps


_CACHED_NC = None


def _get_nc():
    global _CACHED_NC
    if _CACHED_NC is None:
        _CACHED_NC = build_kernel()
    return _CACHED_NC


def run(x, start_padding_indices, trace=False):
    """Run on all 8 cores; returns (out [B, D] f32, BassKernelResults)."""
    nc = _get_nc()
    in_maps = make_host_inputs(x, start_padding_indices)
    res = bass_utils.run_bass_kernel_spmd(
        nc, in_maps, core_ids=list(range(N_CORES)), trace=trace
    )
    outs = [r["out"].reshape(BL, D) for r in res.results]
    return np.concatenate(outs, axis=0), res


def kernel(x, start_padding_indices):
    out, _ = run(x, start_padding_indices, trace=False)
    return out


# revision 28
# speedup vs baseline: 1.0918x; 1.0918x over previous
"""Bass/Trainium2 kernel for nn_AvgPoolBackbone (segment_reduce).

Computes, for each batch row b of x [B, S, D]:
    eff = S if idx[b] == -1 else idx[b]
    out[b] = mean(x[b, :eff], axis=0)   (zeros when eff <= 0)

Strategy
--------
Pure data parallel over 8 NeuronCores (16 batches each).  On the host we
fold the prefix mask AND the 1/eff_len scaling into a single f32 matrix
`maskt` (maskt[p, b*16+k] = (p*16+k < eff[b]) / max(eff[b], 1)) so the
device does no division and no control flow; the masked mean is just a
weighted reduction over the sequence axis.

Per batch, x[b] ([2048, 256] f32, 2 MiB) is DMA'd as [128, 16*256]:
partition p holds the 16 consecutive sequence rows p*16..p*16+15 — one
contiguous 16 KiB DRAM run per partition, which keeps the 16 SDMA
engines at line rate (~435 GB/s aggregate; the kernel is HBM/fabric
bound at ~80 us per core).  One 2 MiB DMA per batch on the sync HWDGE
ring, in consumption order, double-buffered 6 deep.

fp32 TensorE matmuls pay a 2-pass penalty (4 cycles/output element), so
a single engine cannot keep up with the DMA stream in exact fp32.  Each
batch is therefore split across two engines working in parallel:

 - VectorE: 6 of the 16 d-row-slices via a fused multiply-accumulate
   chain, acc_sb[128, d] (+)= x_slice * mask_col
   (scalar_tensor_tensor, per-partition scalar = scaled mask column)
 - TensorE: the other 10 slices as PSUM-accumulated matmuls
   psum[1, d] += mask_col.T @ x_slice, plus one "ones" matmul that
   folds acc_sb across partitions into the same PSUM group.  The fold
   is deferred until the NEXT batch's matmuls are emitted so TensorE
   never stalls at the head of a fresh DVE chain.
 - ScalarE: PSUM -> SBUF result copies (and the small mask-matrix DMA,
   on its own HWDGE ring so the x stream starts immediately).

All arithmetic is exact fp32 (measured rel err vs the f32 reference
~4e-7).  Measured ~101 us per core on TRN2 against a ~80 us DMA floor.
"""

import numpy as np

import concourse.bass as bass
import concourse.tile as tile
from concourse import bacc, mybir
from concourse import bass_utils

F32 = mybir.dt.float32
F32R = mybir.dt.float32r

# Problem config (hardcoded per the harness contract).
B, S, D = 128, 2048, 256
N_CORES = 8
BL = B // N_CORES  # batches per core
P = 128            # SBUF partitions


def build_kernel(bl=BL, s=S, d=D, f32r=False, split=True, bufs=6, q16=6):
    """Build + compile the single-core Bass module (same NEFF on all cores).

    split=True: every batch is split DVE/PE as described in the module
    docstring (exact fp32).  split=False with f32r=True instead runs
    everything on PE in reduced-precision float32r (single-pass matmuls;
    ~5 us faster but ~1.5e-4 rel err).  q16: sixteenths of each batch
    handled by the DVE chain.
    """
    j = s // P  # seq rows per partition (16 at full size)
    mmdt = F32R if f32r else F32
    if f32r:
        split = False
    q = q16 * j // 16  # j-slices per batch on DVE in split mode
    nc = bacc.Bacc("TRN2", target_bir_lowering=False, debug=False)
    x = nc.dram_tensor("x", (bl, s, d), mmdt, kind="ExternalInput")
    maskt = nc.dram_tensor("maskt", (P, bl * j), mmdt, kind="ExternalInput")
    out = nc.dram_tensor("out", (1, bl * d), F32, kind="ExternalOutput")

    with tile.TileContext(nc) as tc:
        with (
            tc.tile_pool(name="xp", bufs=bufs) as xp,
            tc.tile_pool(name="xtp", bufs=1) as xtp,
            tc.tile_pool(name="mp", bufs=1) as mp,
            tc.tile_pool(name="op", bufs=1) as op,
            tc.tile_pool(name="ap", bufs=6) as apool,
            tc.tile_pool(name="ps", bufs=8, space=bass.MemorySpace.PSUM) as ps,
        ):
            m_t = mp.tile([P, bl * j], mmdt)
            # mask load on the scalar HWDGE ring so the sync ring's x
            # stream starts immediately; lands well before first use
            nc.scalar.dma_start(m_t[:], maskt.ap())
            ones_t = None
            if split:
                ones_t = mp.tile([P, 1], F32)
                nc.vector.memset(ones_t[:], 1.0)
            o_t = op.tile([1, bl * d], F32)
            xv = x.ap().rearrange("b (p k) d -> p b (k d)", p=P)

            def dve_chain(b, acc_sb, jis):
                for n, ji in enumerate(jis):
                    xs = x_tiles[b][:, ji * d : (ji + 1) * d]
                    mcol = m_t[:, b * j + ji : b * j + ji + 1]
                    if n == 0:
                        nc.vector.tensor_scalar_mul(acc_sb[:], xs, mcol)
                    else:
                        nc.vector.scalar_tensor_tensor(
                            acc_sb[:],
                            xs,
                            mcol,
                            acc_sb[:],
                            mybir.AluOpType.mult,
                            mybir.AluOpType.add,
                        )

            def pe_mms(b, acc, jis, start, stop):
                for n, ji in enumerate(jis):
                    nc.tensor.matmul(
                        acc[:],
                        m_t[:, b * j + ji : b * j + ji + 1],
                        x_tiles[b][:, ji * d : (ji + 1) * d],
                        start=(start and n == 0),
                        stop=(stop and n == len(jis) - 1),
                    )

            def emit_fold(pb, pacc_sb, pacc):
                nc.tensor.matmul(
                    pacc[:], ones_t[:], pacc_sb[:], start=False, stop=True
                )
                nc.scalar.copy(o_t[:, pb * d : (pb + 1) * d], pacc[:])

            x_tiles = {}
            pending = None  # (batch, acc_sb, acc) awaiting its fold matmul
            for b in range(bl):
                # one 2 MiB DMA per batch on the sync HWDGE ring, in
                # consumption order; lands as [P, j*d] with one contiguous
                # 16 KiB DRAM run per partition.  The two tail batches get
                # dedicated SBUF slots so their DMAs never wait on a slot
                # release gated by late compute.
                if b >= bl - 2:
                    x_t = xtp.tile([P, j * d], mmdt, tag=f"xtail{b}")
                else:
                    x_t = xp.tile([P, j * d], mmdt)
                nc.sync.dma_start(x_t[:], xv[:, b])
                x_tiles[b] = x_t
                if b == bl - 1:
                    # first half of the output ships while the tail computes
                    nc.sync.dma_start(
                        out.ap()[:, : bl * d // 2], o_t[:, : bl * d // 2]
                    )
                if split:
                    acc_sb = apool.tile([P, d], F32)
                    dve_chain(b, acc_sb, range(q))
                    acc = ps.tile([1, d], F32)
                    pe_mms(b, acc, range(q, j), start=True, stop=False)
                    if pending is not None:
                        emit_fold(*pending)
                    pending = (b, acc_sb, acc)
                else:
                    acc = ps.tile([1, d], F32)
                    pe_mms(b, acc, range(j), start=True, stop=True)
                    nc.scalar.copy(o_t[:, b * d : (b + 1) * d], acc[:])
            if pending is not None:
                emit_fold(*pending)
            nc.sync.dma_start(
                out.ap()[:, bl * d // 2 :], o_t[:, bl * d // 2 :]
            )

    nc.compile()
    return nc


def make_host_inputs(x, start_padding_indices, n_cores=N_CORES, bl=BL, s=S, d=D):
    """Shard x and build the per-core scaled mask matrices.

    maskt[p, b*j + ji] = (p*j + ji < eff[b]) / max(eff[b], 1)
    """
    x = np.ascontiguousarray(np.asarray(x, dtype=np.float32))
    idx = np.asarray(start_padding_indices).astype(np.int64)
    j = s // P
    eff = np.where(idx == -1, s, idx).astype(np.int64)  # [B]
    scale = 1.0 / np.maximum(eff, 1).astype(np.float64)
    mask = (np.arange(s)[None, :] < eff[:, None]) * scale[:, None]  # [B, S] f64
    mask = mask.astype(np.float32)
    # [B, S] -> [B, P, j] (s-major within partition) -> cores pack [P, bl*j]
    mask_pj = mask.reshape(-1, P, j)  # [B, P, j]
    in_maps = []
    for c in range(n_cores):
        mb = mask_pj[c * bl : (c + 1) * bl]  # [bl, P, j]
        maskt = np.ascontiguousarray(mb.transpose(1, 0, 2).reshape(P, bl * j))
        in_maps.append(
            {
                "x": np.ascontiguousarray(x[c * bl : (c + 1) * bl]),
                "maskt": maskt,
            }
        )
    return in_maps


_CACHED_NC = None


def _get_nc():
    global _CACHED_NC
    if _CACHED_NC is None:
        _CACHED_NC = build_kernel()
    return _CACHED_NC


def run(x, start_padding_indices, trace=False):
    """Run on all 8 cores; returns (out [B, D] f32, BassKernelResults)."""
    nc = _get_nc()
    in_maps = make_host_inputs(x, start_padding_indices)
    res = bass_utils.run_bass_kernel_spmd(
        nc, in_maps, core_ids=list(range(N_CORES)), trace=trace
    )
    outs = [r["out"].reshape(BL, D) for r in res.results]
    return np.concatenate(outs, axis=0), res


def kernel(x, start_padding_indices):
    out, _ = run(x, start_padding_indices, trace=False)
    return out


# revision 29
# speedup vs baseline: 1.1506x; 1.0539x over previous
"""Bass/Trainium2 kernel for nn_AvgPoolBackbone (segment_reduce).

Computes, for each batch row b of x [B, S, D]:
    eff = S if idx[b] == -1 else idx[b]
    out[b] = mean(x[b, :eff], axis=0)   (zeros when eff <= 0)

Strategy
--------
Pure data parallel over 8 NeuronCores (16 batches each).  On the host we
fold the prefix mask AND the 1/eff_len scaling into a single f32 matrix
`maskt` (maskt[p, b*16+k] = (p*16+k < eff[b]) / max(eff[b], 1)) so the
device does no division and no control flow; the masked mean is just a
weighted reduction over the sequence axis.

Per batch, x[b] ([2048, 256] f32, 2 MiB) is DMA'd as [128, 16*256]:
partition p holds the 16 consecutive sequence rows p*16..p*16+15 — one
contiguous 16 KiB DRAM run per partition, which keeps the 16 SDMA
engines at line rate (~435 GB/s aggregate; the kernel is HBM/fabric
bound at ~80 us per core).  One 2 MiB DMA per batch on the sync HWDGE
ring, in consumption order, double-buffered 6 deep.

fp32 TensorE matmuls pay a 2-pass penalty (4 cycles/output element), so
a single engine cannot keep up with the DMA stream in exact fp32.  Each
batch is therefore split across two engines working in parallel:

 - VectorE: 6 of the 16 d-row-slices via a fused multiply-accumulate
   chain, acc_sb[128, d] (+)= x_slice * mask_col
   (scalar_tensor_tensor, per-partition scalar = scaled mask column)
 - TensorE: the other 10 slices as PSUM-accumulated matmuls
   psum[1, d] += mask_col.T @ x_slice, plus one "ones" matmul that
   folds acc_sb across partitions into the same PSUM group.  The fold
   is deferred until the NEXT batch's matmuls are emitted so TensorE
   never stalls at the head of a fresh DVE chain.
 - ScalarE: PSUM -> SBUF result copies (and the small mask-matrix DMA,
   on its own HWDGE ring so the x stream starts immediately).

All arithmetic is exact fp32 (measured rel err vs the f32 reference
~4e-7).  Measured ~101 us per core on TRN2 against a ~80 us DMA floor.
"""

import numpy as np

import concourse.bass as bass
import concourse.tile as tile
from concourse import bacc, mybir
from concourse import bass_utils

F32 = mybir.dt.float32
F32R = mybir.dt.float32r

# Problem config (hardcoded per the harness contract).
B, S, D = 128, 2048, 256
N_CORES = 8
BL = B // N_CORES  # batches per core
P = 128            # SBUF partitions


def build_kernel(bl=BL, s=S, d=D, f32r=False, split=True, bufs=6, q16=6, g=0):
    """Build + compile the single-core Bass module (same NEFF on all cores).

    split=True: every batch is split DVE/PE as described in the module
    docstring (exact fp32).  split=False with f32r=True instead runs
    everything on PE in reduced-precision float32r (single-pass matmuls;
    ~5 us faster but ~1.5e-4 rel err).  q16: sixteenths of each batch
    handled by the DVE chain.
    """
    j = s // P  # seq rows per partition (16 at full size)
    mmdt = F32R if f32r else F32
    if f32r:
        split = False
    q = q16 * j // 16  # j-slices per batch on DVE in split mode
    nc = bacc.Bacc("TRN2", target_bir_lowering=False, debug=False)
    x = nc.dram_tensor("x", (bl, s, d), mmdt, kind="ExternalInput")
    maskt = nc.dram_tensor("maskt", (P, bl * j), mmdt, kind="ExternalInput")
    out = nc.dram_tensor("out", (1, bl * d), F32, kind="ExternalOutput")

    with tile.TileContext(nc) as tc:
        with (
            tc.tile_pool(name="xp", bufs=bufs) as xp,
            tc.tile_pool(name="xtp", bufs=1) as xtp,
            tc.tile_pool(name="mp", bufs=1) as mp,
            tc.tile_pool(name="op", bufs=1) as op,
            tc.tile_pool(name="ap", bufs=6) as apool,
            tc.tile_pool(name="ps", bufs=8, space=bass.MemorySpace.PSUM) as ps,
        ):
            m_t = mp.tile([P, bl * j], mmdt)
            # mask load on the scalar HWDGE ring so the sync ring's x
            # stream starts immediately; lands well before first use
            nc.scalar.dma_start(m_t[:], maskt.ap())
            ones_t = None
            if split:
                ones_t = mp.tile([P, 1], F32)
                nc.vector.memset(ones_t[:], 1.0)
            o_t = op.tile([1, bl * d], F32)
            xv = x.ap().rearrange("b (p k) d -> p b (k d)", p=P)

            def dve_chain(b, acc_sb, jis, eng=None):
                eng = eng or nc.vector
                for n, ji in enumerate(jis):
                    xs = x_tiles[b][:, ji * d : (ji + 1) * d]
                    mcol = m_t[:, b * j + ji : b * j + ji + 1]
                    if n == 0:
                        eng.tensor_scalar_mul(acc_sb[:], xs, mcol)
                    else:
                        eng.scalar_tensor_tensor(
                            acc_sb[:],
                            xs,
                            mcol,
                            acc_sb[:],
                            mybir.AluOpType.mult,
                            mybir.AluOpType.add,
                        )

            def pe_mms(b, acc, jis, start, stop):
                for n, ji in enumerate(jis):
                    nc.tensor.matmul(
                        acc[:],
                        m_t[:, b * j + ji : b * j + ji + 1],
                        x_tiles[b][:, ji * d : (ji + 1) * d],
                        start=(start and n == 0),
                        stop=(stop and n == len(jis) - 1),
                    )

            def emit_fold(pb, paccs, pacc):
                for n, a in enumerate(paccs):
                    nc.tensor.matmul(
                        pacc[:], ones_t[:], a[:],
                        start=False, stop=(n == len(paccs) - 1),
                    )
                nc.scalar.copy(o_t[:, pb * d : (pb + 1) * d], pacc[:])

            x_tiles = {}
            pending = None  # (batch, acc_sb, acc) awaiting its fold matmul
            for b in range(bl):
                # one 2 MiB DMA per batch on the sync HWDGE ring, in
                # consumption order; lands as [P, j*d] with one contiguous
                # 16 KiB DRAM run per partition.  The two tail batches get
                # dedicated SBUF slots so their DMAs never wait on a slot
                # release gated by late compute.
                if b >= bl - 2:
                    x_t = xtp.tile([P, j * d], mmdt, tag=f"xtail{b}")
                else:
                    x_t = xp.tile([P, j * d], mmdt)
                nc.sync.dma_start(x_t[:], xv[:, b])
                x_tiles[b] = x_t
                if b == bl - 1:
                    # first half of the output ships while the tail computes
                    nc.sync.dma_start(
                        out.ap()[:, : bl * d // 2], o_t[:, : bl * d // 2]
                    )
                if split:
                    acc_sb = apool.tile([P, d], F32)
                    dve_chain(b, acc_sb, range(q))
                    accs = [acc_sb]
                    gq = min(g, j - q) if j > 2 else 0
                    if gq > 0:
                        acc_g = apool.tile([P, d], F32)
                        dve_chain(b, acc_g, range(q, q + gq), eng=nc.gpsimd)
                        accs.append(acc_g)
                    acc = ps.tile([1, d], F32)
                    pe_mms(b, acc, range(q + gq, j), start=True, stop=False)
                    if pending is not None:
                        emit_fold(*pending)
                    pending = (b, accs, acc)
                else:
                    acc = ps.tile([1, d], F32)
                    pe_mms(b, acc, range(j), start=True, stop=True)
                    nc.scalar.copy(o_t[:, b * d : (b + 1) * d], acc[:])
            if pending is not None:
                emit_fold(*pending)
            nc.sync.dma_start(
                out.ap()[:, bl * d // 2 :], o_t[:, bl * d // 2 :]
            )

    nc.compile()
    return nc


def make_host_inputs(x, start_padding_indices, n_cores=N_CORES, bl=BL, s=S, d=D):
    """Shard x and build the per-core scaled mask matrices.

    maskt[p, b*j + ji] = (p*j + ji < eff[b]) / max(eff[b], 1)
    """
    x = np.ascontiguousarray(np.asarray(x, dtype=np.float32))
    idx = np.asarray(start_padding_indices).astype(np.int64)
    j = s // P
    eff = np.where(idx == -1, s, idx).astype(np.int64)  # [B]
    scale = 1.0 / np.maximum(eff, 1).astype(np.float64)
    mask = (np.arange(s)[None, :] < eff[:, None]) * scale[:, None]  # [B, S] f64
    mask = mask.astype(np.float32)
    # [B, S] -> [B, P, j] (s-major within partition) -> cores pack [P, bl*j]
    mask_pj = mask.reshape(-1, P, j)  # [B, P, j]
    in_maps = []
    for c in range(n_cores):
        mb = mask_pj[c * bl : (c + 1) * bl]  # [bl, P, j]
        maskt = np.ascontiguousarray(mb.transpose(1, 0, 2).reshape(P, bl * j))
        in_maps.append(
            {
                "x": np.ascontiguousarray(x[c * bl : (c + 1) * bl]),
                "maskt": maskt,
            }
        )
    return in_maps


_CACHED_NC = None


def _get_nc():
    global _CACHED_NC
    if _CACHED_NC is None:
        _CACHED_NC = build_kernel()
    return _CACHED_NC


def run(x, start_padding_indices, trace=False):
    """Run on all 8 cores; returns (out [B, D] f32, BassKernelResults)."""
    nc = _get_nc()
    in_maps = make_host_inputs(x, start_padding_indices)
    res = bass_utils.run_bass_kernel_spmd(
        nc, in_maps, core_ids=list(range(N_CORES)), trace=trace
    )
    outs = [r["out"].reshape(BL, D) for r in res.results]
    return np.concatenate(outs, axis=0), res


def kernel(x, start_padding_indices):
    out, _ = run(x, start_padding_indices, trace=False)
    return out
